# revision 2
# baseline (speedup 1.0000x reference)
"""Distributed embedding lookup v2: dma_gather + pre-scale + dma_scatter_add.

Sharding: output rows split contiguously over 8 cores (53,248 rows each).
Each core's keys are routed (host side) into buckets keyed by
(output-row half, vocab segment of 32768 rows) so that:
  - dma_gather's int16 indices address rows within one table segment,
  - dma_scatter_add's int16 indices address rows within one output half.
Pad slots use a valid gather index (0) and scatter into per-half dummy rows
with recip=0, so every instruction's index count is compile-time static and
the one NEFF is shared by all 8 cores.  The mean combine is folded into a
per-key 1/count pre-scale on the Vector engine between gather and scatter.
"""

import numpy as np

CORES = 8
SEG = 32768  # vocab rows per gather segment (int16 index range)
HALF_PAD = 128  # dummy scatter rows appended per output half


def _build_program(V, D, HR, bucket_sizes, MAXC):
    """HR: real rows per half. bucket_sizes: list of (half, seg, size) with
    static padded size (multiple of 128). MAXC = max chunks (=size//128)."""
    import concourse.bacc as bacc
    import concourse.mybir as mybir
    import concourse.tile as tile

    dt = mybir.dt
    TOT = max(off + sz for _, _, off, sz, _l in bucket_sizes)
    HOUT = HR + HALF_PAD

    nc = bacc.Bacc("TRN2", target_bir_lowering=False, debug=False)
    table = nc.dram_tensor("table", [V, D], dt.float32, kind="ExternalInput").ap()
    gidx = nc.dram_tensor("gidx", [128, TOT // 16], dt.int16, kind="ExternalInput").ap()
    sidx = nc.dram_tensor("sidx", [128, TOT // 16], dt.int16, kind="ExternalInput").ap()
    recip = nc.dram_tensor("recip", [128, TOT // 128], dt.float32, kind="ExternalInput").ap()
    out0 = nc.dram_tensor("out0", [HOUT, D], dt.float32, kind="ExternalOutput").ap()
    out1 = nc.dram_tensor("out1", [HOUT, D], dt.float32, kind="ExternalOutput").ap()
    outs = (out0, out1)

    with tile.TileContext(nc) as tc:
        with (
            tc.tile_pool(name="const", bufs=1) as cpool,
            tc.tile_pool(name="gb", bufs=4) as gpool,
        ):
            gi_sb = cpool.tile([128, TOT // 16], dt.int16, tag="gi")
            si_sb = cpool.tile([128, TOT // 16], dt.int16, tag="si")
            rc_sb = cpool.tile([128, TOT // 128], dt.float32, tag="rc")
            nc.sync.dma_start(gi_sb[:], gidx[:])
            nc.sync.dma_start(si_sb[:], sidx[:])
            nc.sync.dma_start(rc_sb[:], recip[:])

            for h, s, off, sz, levs in bucket_sizes:
                nchunk = sz // 128
                seg_rows = min(SEG, V - s * SEG)
                G = gpool.tile([128, MAXC, D], dt.float32, tag="g", name="g")
                nc.gpsimd.dma_gather(
                    out_ap=G[:, :nchunk, :],
                    in_ap=table[s * SEG : s * SEG + seg_rows, :],
                    idxs_ap=gi_sb[:, off // 16 : (off + sz) // 16],
                    num_idxs=sz,
                    num_idxs_reg=sz,
                    elem_size=D,
                    queue_num=0,
                    single_packet=False,
                )
                rc_bc = rc_sb[:, off // 128 : (off + sz) // 128].to_broadcast(
                    [128, nchunk, D]
                )
                nc.vector.tensor_tensor(
                    out=G[:, :nchunk, :],
                    in0=G[:, :nchunk, :],
                    in1=rc_bc,
                    op=mybir.AluOpType.mult,
                )
                for loff, lsz in levs:
                    c0 = (loff - off) // 128
                    nc.gpsimd.dma_scatter_add(
                        out_ap=outs[h][:],
                        in_ap=G[:, c0 : c0 + lsz // 128, :],
                        idxs_ap=si_sb[:, loff // 16 : (loff + lsz) // 16],
                        num_idxs=lsz,
                        num_idxs_reg=lsz,
                        elem_size=D,
                        queue_num=0,
                        single_packet=False,
                    )

    nc.compile()
    return nc


def _pack(vals, rows, V, ROWS, cnt):
    """Bucket keys by (core, half, segment); pad to per-position max over
    cores. Returns device arrays + bucket size table."""
    RPC = ROWS // CORES
    HR = RPC // 2
    NSEG = -(-V // SEG)
    core = rows // RPC
    lr = rows - core * RPC
    h = lr // HR
    s = vals // SEG
    # level = rank of key within (core, half, segment, row) so that each
    # scatter instruction (bucket) carries unique output rows (the HW CCE
    # add races on concurrent duplicates; Tile serializes across instrs).
    key1 = ((core * 2 + h) * NSEG + s) * np.int64(ROWS) + lr
    o1 = np.argsort(key1, kind="stable")
    ks = key1[o1]
    newrun = np.concatenate([[True], ks[1:] != ks[:-1]])
    runid = np.cumsum(newrun) - 1
    runstart = np.flatnonzero(newrun)
    lev_sorted = np.arange(len(ks)) - runstart[runid]
    lev = np.empty(len(ks), dtype=np.int64)
    lev[o1] = lev_sorted  # uncapped: every duplicate gets its own level
    NLEV = int(lev.max()) + 1
    bid = ((core * 2 + h) * NSEG + s) * NLEV + lev
    NB = CORES * 2 * NSEG * NLEV
    order = np.argsort(bid * np.int64(V + 1) + vals, kind="stable")  # bucket, then key
    bc = np.bincount(bid, minlength=NB).reshape(CORES, 2 * NSEG * NLEV)
    sz = -(-bc.max(axis=0) // 128) * 128  # static per-position sizes [2*NSEG]
    TOT = int(sz.sum())

    starts = np.zeros(2 * NSEG * NLEV + 1, dtype=np.int64)
    np.cumsum(sz, out=starts[1:])

    gi = np.zeros((CORES, TOT), np.int16)
    si = np.zeros((CORES, TOT), np.int16)
    rc = np.zeros((CORES, TOT), np.float32)

    vs, ls, hs, ss, cs, bs = (
        vals[order], lr[order], h[order], s[order], core[order], bid[order],
    )
    pos_in_b = np.arange(len(order)) - np.concatenate(
        [[0], np.cumsum(np.bincount(bs, minlength=NB))]
    )[bs]
    slot = starts[bs % (2 * NSEG * NLEV)] + pos_in_b
    gi[cs, slot] = (vs - ss * SEG).astype(np.int16)
    dummy = HR + (slot % HALF_PAD)
    si_real = (ls - hs * HR).astype(np.int16)
    si[:, :] = 0
    # initialize pads: gather idx 0 (valid), scatter to dummy rows, recip 0
    for b in range(2 * NSEG * NLEV):
        si[:, starts[b] : starts[b + 1]] = (
            HR + (np.arange(sz[b]) % HALF_PAD)
        ).astype(np.int16)
    si[cs, slot] = si_real
    rc[cs, slot] = (1.0 / np.maximum(cnt, 1.0))[rows[order]].astype(np.float32)

    # 16-wrap each bucket independently: [SZ] -> [16, SZ//16] -> tile to 128
    def wrap16(a):
        outp = np.empty((CORES, 128, TOT // 16), a.dtype)
        for b in range(2 * NSEG * NLEV):
            seg16 = a[:, starts[b] : starts[b + 1]].reshape(CORES, sz[b] // 16, 16)
            outp[:, :, starts[b] // 16 : starts[b + 1] // 16] = np.tile(
                seg16.transpose(0, 2, 1), (1, 8, 1)
            )
        return outp

    gi_w = wrap16(gi)
    si_w = wrap16(si)
    # recip layout matches G: key at bucket slot i -> (i%128, chunks_off + i//128)
    rc_dev = np.empty((CORES, 128, TOT // 128), np.float32)
    for b in range(2 * NSEG * NLEV):
        seg = rc[:, starts[b] : starts[b + 1]].reshape(CORES, sz[b] // 128, 128)
        rc_dev[:, :, starts[b] // 128 : starts[b + 1] // 128] = seg.transpose(0, 2, 1)

    # merge levels of one (half, segment): their slot ranges are adjacent
    # (bucket index = (h*NSEG + s)*NLEV + lev), so one gather + one prescale
    # covers them; only the scatters stay split per level (unique rows each).
    bucket_sizes = []
    for hh in range(2):
        for seg in range(NSEG):
            b0 = (hh * NSEG + seg) * NLEV
            levs = [
                (int(starts[b0 + l]), int(sz[b0 + l]))
                for l in range(NLEV)
                if sz[b0 + l] > 0
            ]
            if not levs:
                continue
            off = levs[0][0]
            tot = sum(l for _, l in levs)
            bucket_sizes.append((hh, seg, off, tot, levs))
    bucket_sizes.sort(key=lambda e: (e[1], e[0]))  # segment-major, halves interleaved
    return gi_w, si_w, rc_dev, bucket_sizes, HR


def kernel(table, values, row_indices):
    from concourse.bass_utils import run_bass_kernel_spmd

    table = np.ascontiguousarray(np.asarray(table), dtype=np.float32)
    vals = np.asarray(values).astype(np.int64)
    rows = np.asarray(row_indices).astype(np.int64)
    V, D = table.shape
    B, S = 16384, 26
    ROWS = B * S
    cnt = np.bincount(rows, minlength=ROWS).astype(np.float32)

    gi_w, si_w, rc_dev, bucket_sizes, HR = _pack(vals, rows, V, ROWS, cnt)
    MAXC = max(sz for _, _, _, sz, _l in bucket_sizes) // 128
    nc = _build_program(V, D, HR, bucket_sizes, MAXC)

    in_maps = [
        {
            "table": table,
            "gidx": np.ascontiguousarray(gi_w[c]),
            "sidx": np.ascontiguousarray(si_w[c]),
            "recip": np.ascontiguousarray(rc_dev[c]),
        }
        for c in range(CORES)
    ]
    res = run_bass_kernel_spmd(nc, in_maps, core_ids=list(range(CORES)))
    global _last_results
    _last_results = res
    HOUT = HR + HALF_PAD
    outs = [
        np.concatenate(
            [np.asarray(res.results[c]["out0"])[:HR], np.asarray(res.results[c]["out1"])[:HR]],
            axis=0,
        )
        for c in range(CORES)
    ]
    full = np.concatenate(outs, axis=0)
    return np.ascontiguousarray(full.reshape(B, S, D), dtype=np.float32)

